# revision 1
# baseline (speedup 1.0000x reference)
"""Fused ViT transformer block on Trainium2, data-parallel over batch across 8 cores.

Per-core kernel (batch shard [4, 577, 1024]) computes the whole block:
  x + Attn(LN1(x)) -> x2 ; x2 + MLP(LN2(x2))
with all activations resident in SBUF (no DRAM round-trips for intermediates).

Matmuls run in float32r (fp32 storage, single-pass reduced-precision PE mode,
4x the fp32 streaming rate). fp32r ISA restrictions shape the tiling:
  - lhsT free size (M) must be exactly 128 -> token tiles on the PSUM partition
    axis are overlapped 128-row windows (the tail tile recomputes a few rows and
    evicts only its fresh rows).
  - moving-operand / PSUM free size (N) must be even, PSUM start 8B-aligned ->
    token col-blocks are even-sized with a 1..3-column overlap; the key axis is
    padded to even width with a zeroed pad column that softmax never reads.

Layouts:
  - LayerNorms run in token-major [t, D] tiles (free-dim reductions, bn_stats),
    then PE-transpose (fp32) into feature-major xnT [D, t] which feeds every
    matmul with the contraction dim on partitions.
  - QKV per head pair (2 heads = 128 rows) -> qT/kT/vT [128, T]; scores in PSUM
    [q, k]; exp on ACT with fused 1/sqrt(hd) scale and accumulated row sums;
    rows normalized in SBUF; PE-transposed; AV accumulates attn_outT [D, T]
    (full-128 lhsT, only the current head's 64 rows evicted - N-cycle pricing
    makes the extra rows free).
  - proj/fc1/fc2 are tiled matmuls; fc2 accumulates into an SBUF accumulator
    initialized with x2 + fc2_bias, so the final residual is free. Hidden
    blocks are processed in pairs to halve the accumulator traffic.

Host-side precompute folds LN affine params into the adjacent weights
(w' = g[:,None]*w, c = b@w + bias) and the proj bias into the residual stream.
"""

import numpy as np
from contextlib import ExitStack

import concourse.bass as bass
import concourse.mybir as mybir
import concourse.tile as tile
from concourse import bacc
from concourse.masks import make_identity

P = 128
F32 = mybir.dt.float32
FR = mybir.dt.float32r
BF = mybir.dt.bfloat16
AF = mybir.ActivationFunctionType
ALU = mybir.AluOpType


def _chunks(total, step):
    return [(i, min(step, total - i)) for i in range(0, total, step)]


def _even_blocks(total, maxn=512):
    """Even-sized blocks covering [0, total); the last block overlaps its
    predecessor when needed. Every block size is even (fp32r N requirement)."""
    import math
    n = math.ceil(total / maxn)
    size = math.ceil(total / n)
    size += size % 2
    assert size <= maxn and size <= total
    return [(i * size, size) for i in range(n - 1)] + [(total - size, size)]


def build_nc(B=4, T=577, D=1024, H=16, HD=64, HID=4096, eps=1e-6,
             attn_bf16=True):
    """Build the single-core Bass program for a [B, T, D] shard."""
    assert D % P == 0 and H % 2 == 0 and HD == 64 and HID % 512 == 0
    assert T >= P
    KD = D // P              # k-tiles over model dim
    NPAIR = H // 2           # head pairs
    TOK = B * T
    HB = 512                 # hidden block
    NHB = HID // HB
    assert NHB % 2 == 0
    KH = HB // P             # k-tiles per hidden block (4)
    NHT = HID // P           # total fc1 output tiles
    scale = 1.0 / float(np.sqrt(HD))

    t_tiles = _chunks(T, P)          # non-overlapping token tiles (LN1 load)
    NTT = len(t_tiles)
    k_chunks = t_tiles               # key chunks
    # overlapped 128-row token windows for fp32r M=128 matmuls
    t_tiles_ov = [(min(i * P, T - P), P) for i in range(NTT)]
    fresh = []                       # first non-overlapped row of each window
    pe = 0
    for (t0, _) in t_tiles_ov:
        fresh.append(pe - t0)
        pe = t0 + P
    n_blocks_T = _even_blocks(T)     # even token col-blocks (rhs free dim)
    d_blocks = _chunks(D, 512)       # output col blocks (512 | D)
    kb_blocks = _chunks(T, 512)      # key/query col blocks (PSUM bank-aligned);
    assert T <= 1024                 # odd tails run as plain fp32 matmuls

    AT = BF if attn_bf16 else F32      # attention-internal dtype
    nc = bacc.Bacc(None, target_bir_lowering=False, debug=False)

    x_d = nc.dram_tensor("x", [TOK, D], F32, kind="ExternalInput")
    xb_d = nc.dram_tensor("xb", [TOK, D], F32, kind="ExternalInput")
    wqkv_d = nc.dram_tensor("wqkv", [D, NPAIR, 3 * P], FR, kind="ExternalInput")
    cq_d = nc.dram_tensor("cq", [P, NPAIR * 3], F32, kind="ExternalInput")
    wp_d = nc.dram_tensor("wproj", [D, D], FR, kind="ExternalInput")
    wf1_d = nc.dram_tensor("wfc1", [D, HID], FR, kind="ExternalInput")
    cf1_d = nc.dram_tensor("cf1", [P, NHT], F32, kind="ExternalInput")
    wf2_d = nc.dram_tensor("wfc2", [HID, D], FR, kind="ExternalInput")
    cf2_d = nc.dram_tensor("cf2", [D], F32, kind="ExternalInput")
    out_d = nc.dram_tensor("out", [TOK, D], F32, kind="ExternalOutput")

    with tile.TileContext(nc) as tc, ExitStack() as ctx:
        const = ctx.enter_context(tc.tile_pool(name="const", bufs=1))
        xpool = ctx.enter_context(tc.tile_pool(name="xin", bufs=4))
        statp = ctx.enter_context(tc.tile_pool(name="stat", bufs=10))
        xn1T_p = ctx.enter_context(tc.tile_pool(name="xn1T", bufs=1))
        wq_p = ctx.enter_context(tc.tile_pool(name="wq", bufs=9))
        qkvt_p = ctx.enter_context(tc.tile_pool(name="qkvt", bufs=2))
        sA_p = ctx.enter_context(tc.tile_pool(name="attnS", bufs=7))
        tA_p = ctx.enter_context(tc.tile_pool(name="attnT", bufs=7))
        vkd_p = ctx.enter_context(tc.tile_pool(name="vkd", bufs=3))
        aT_p = ctx.enter_context(tc.tile_pool(name="aT", bufs=1))
        w5_p = ctx.enter_context(tc.tile_pool(name="w512", bufs=9))
        xb_p = ctx.enter_context(tc.tile_pool(name="xbp", bufs=6))
        xn2T_p = ctx.enter_context(tc.tile_pool(name="xn2T", bufs=1))
        oacc_p = ctx.enter_context(tc.tile_pool(name="oacc", bufs=1))
        hT_p = ctx.enter_context(tc.tile_pool(name="hT", bufs=2))
        psum = ctx.enter_context(tc.tile_pool(name="psum", bufs=2, space="PSUM"))
        psum1 = ctx.enter_context(tc.tile_pool(name="psum1", bufs=2, space="PSUM"))

        ident = const.tile([P, P], F32)
        make_identity(nc, ident)
        if attn_bf16:
            identA = const.tile([P, P], AT)
            nc.vector.tensor_copy(out=identA[:, :], in_=ident[:, :])
        else:
            identA = ident
        eps_t = const.tile([P, 1], F32)
        nc.vector.memset(eps_t, eps)
        cq_sb = const.tile([P, NPAIR * 3], F32)
        nc.sync.dma_start(out=cq_sb[:, :], in_=cq_d[:, :])
        cf1_sb = const.tile([P, NHT], F32)
        nc.sync.dma_start(out=cf1_sb[:, :], in_=cf1_d[:, :])
        cf2_rep = const.tile([P, D], F32)
        cf2_ap = cf2_d[:]
        nc.sync.dma_start(
            out=cf2_rep[:, :],
            in_=bass.AP(tensor=cf2_ap.tensor, offset=cf2_ap.offset,
                        ap=[[0, P]] + list(cf2_ap.ap)),
        )

        def layernorm_transpose(src_tiles, dstT):
            """src_tiles: list of (tile, t0, r0, rows) token-major tiles holding
            tokens t0..t0+(rows-r0) in partition rows r0..rows. In-place
            normalize, then PE-transpose into dstT [P, KD, T]."""
            for (xt, t0, r0, rows) in src_tiles:
                nsub = max(1, D // 512)
                stats = statp.tile([P, nsub, 6], F32, tag="bnst")
                xv = xt.rearrange("p (s f) -> p s f", s=nsub)
                for s in range(nsub):
                    nc.vector.bn_stats(out=stats[r0:rows, s, :], in_=xv[r0:rows, s, :])
                mv = statp.tile([P, 2], F32, tag="mv")
                nc.vector.bn_aggr(out=mv[r0:rows, :], in_=stats[r0:rows])
                istd = statp.tile([P, 1], F32, tag="istd")
                nc.scalar.activation(out=istd[r0:rows], in_=mv[r0:rows, 1:2],
                                     func=AF.Sqrt, bias=eps_t[r0:rows])
                nc.vector.reciprocal(out=istd[r0:rows], in_=istd[r0:rows])
                nc.vector.tensor_scalar(
                    out=xt[r0:rows, :], in0=xt[r0:rows, :],
                    scalar1=mv[r0:rows, 0:1], scalar2=istd[r0:rows],
                    op0=ALU.subtract, op1=ALU.mult,
                )
                nt = rows - r0
                for kt0 in range(0, KD, 4):
                    ng = min(4, KD - kt0)
                    trp = psum.tile([P, ng, nt], F32, tag="tr")
                    for j in range(ng):
                        kt = kt0 + j
                        nc.tensor.matmul(
                            trp[:, j, :], xt[r0:rows, kt * P:(kt + 1) * P],
                            ident[r0:rows, r0:rows], is_transpose=True,
                            start=(j == 0), stop=(j == ng - 1),
                        )
                    nc.vector.tensor_copy(
                        out=dstT[:, kt0:kt0 + ng, t0:t0 + nt], in_=trp[:, :, :])

        for g in range(B):
            # ---------------- Phase A: LN1 ----------------
            xn1T = xn1T_p.tile([P, KD, T], FR)
            ln_tiles = []
            for (t0, tsz) in t_tiles:
                xt = xpool.tile([P, D], F32)
                nc.sync.dma_start(out=xt[:tsz, :], in_=x_d[g * T + t0:g * T + t0 + tsz, :])
                ln_tiles.append((xt, t0, 0, tsz))
            layernorm_transpose(ln_tiles, xn1T)

            # ---------------- Phase B: QKV + attention, per head pair ----------------
            aT = aT_p.tile([P, KD, T], FR)
            for p in range(NPAIR):
                wts = []
                for kt in range(KD):
                    wt = wq_p.tile([P, 3 * P], FR)
                    nc.sync.dma_start(out=wt[:, :], in_=wqkv_d[kt * P:(kt + 1) * P, p, :])
                    wts.append(wt)
                qT = qkvt_p.tile([P, T], FR, tag="tq")
                kT = qkvt_p.tile([P, T], FR, tag="tk")
                vT = qkvt_p.tile([P, T], AT, tag="tv")
                for i, dst in enumerate((qT, kT, vT)):
                    for (n0, nsz) in n_blocks_T:
                        ps = psum.tile([P, 512], F32, tag="mm")
                        for kt in range(KD):
                            nc.tensor.matmul(
                                ps[:, :nsz], wts[kt][:, i * P:(i + 1) * P],
                                xn1T[:, kt, n0:n0 + nsz],
                                start=(kt == 0), stop=(kt == KD - 1),
                            )
                        nc.scalar.add(dst[:, n0:n0 + nsz], ps[:, :nsz],
                                      cq_sb[:, p * 3 + i:p * 3 + i + 1])
                # V -> [k, d] layout for AV lhsT (both heads at once)
                vkd = vkd_p.tile([P, NTT, P], FR if not attn_bf16 else BF)
                for kti, (k0, ksz) in enumerate(k_chunks):
                    trp = psum.tile([P, P], AT, tag="tr")
                    nc.tensor.matmul(trp[:ksz, :], vT[:, k0:k0 + ksz], identA[:, :],
                                     is_transpose=True)
                    nc.vector.tensor_copy(out=vkd[:ksz, kti, :], in_=trp[:ksz, :])

                for hi in range(2):
                    hb0 = 64 * hi
                    sA_tiles = []
                    for (q0, _) in t_tiles_ov:
                        sp = psum1.tile([P, 1024], F32, tag="scav")
                        for (k0, ksz) in kb_blocks:
                            qs = qT[hb0:hb0 + 64, q0:q0 + P]
                            ks = kT[hb0:hb0 + 64, k0:k0 + ksz]
                            if ksz % 2:
                                qs, ks = qs.bitcast(F32), ks.bitcast(F32)
                            nc.tensor.matmul(sp[:, k0:k0 + ksz], qs, ks)
                        sA = sA_p.tile([P, T], AT)
                        lsum = statp.tile([P, 1], F32, tag="lsum")
                        nc.scalar.activation(sA[:, :], sp[:, :T], func=AF.Exp,
                                             scale=scale, accum_out=lsum[:])
                        linv = statp.tile([P, 1], F32, tag="linv")
                        nc.vector.reciprocal(out=linv[:], in_=lsum[:])
                        nc.vector.tensor_scalar_mul(out=sA[:, :], in0=sA[:, :],
                                                    scalar1=linv[:])
                        sA_tiles.append((sA, q0))
                    # transpose attn [q, k] -> [k, q]
                    tA_tiles = []
                    for kti, (k0, ksz) in enumerate(k_chunks):
                        tA = tA_p.tile([P, T], FR if not attn_bf16 else BF)
                        groups = []
                        cur = [sA_tiles[0]]
                        for e in sA_tiles[1:]:
                            if e[1] == cur[-1][1] + P and len(cur) < 4:
                                cur.append(e)
                            else:
                                groups.append(cur)
                                cur = [e]
                        groups.append(cur)
                        for grp in groups:
                            qb0 = grp[0][1]
                            qtot = (grp[-1][1] + P) - qb0
                            trp = psum.tile([P, 512], AT, tag="tr")
                            for j, (sA, q0) in enumerate(grp):
                                nc.tensor.matmul(
                                    trp[:ksz, q0 - qb0:q0 - qb0 + P],
                                    sA[:, k0:k0 + ksz], identA[:, :],
                                    is_transpose=True,
                                    start=(j == 0), stop=(j == len(grp) - 1),
                                )
                            nc.vector.tensor_copy(out=tA[:ksz, qb0:qb0 + qtot],
                                                  in_=trp[:ksz, :qtot])
                        tA_tiles.append(tA)
                    # AV: full-128 lhsT; only this head's 64 rows are evicted
                    av = psum1.tile([P, 1024], F32, tag="scav")
                    for (qb0, qbsz) in kb_blocks:
                        for kti, (k0, ksz) in enumerate(k_chunks):
                            vs = vkd[:ksz, kti, :]
                            ts = tA_tiles[kti][:ksz, qb0:qb0 + qbsz]
                            if qbsz % 2 and not attn_bf16:
                                vs, ts = vs.bitcast(F32), ts.bitcast(F32)
                            nc.tensor.matmul(
                                av[:, qb0:qb0 + qbsz], vs, ts,
                                start=(kti == 0), stop=(kti == NTT - 1),
                            )
                    nc.scalar.copy(aT[hb0:hb0 + 64, p, :], av[hb0:hb0 + 64, :T])

            # ---------------- Phase C: proj + residual + LN2 ----------------
            # x2 is accumulated in-place into the xb residual tiles (overlapped
            # 128-row windows; only fresh rows are written/used).
            x2s = []
            for (o0, osz) in d_blocks:
                wps = []
                for kt in range(KD):
                    wt = w5_p.tile([P, 512], FR, tag="w")
                    nc.sync.dma_start(out=wt[:, :osz], in_=wp_d[kt * P:(kt + 1) * P, o0:o0 + osz])
                    wps.append(wt)
                for tt, (t0, _) in enumerate(t_tiles_ov):
                    if o0 == 0:
                        xbt = xb_p.tile([P, D], F32)
                        nc.sync.dma_start(
                            out=xbt[:, :],
                            in_=xb_d[g * T + t0:g * T + t0 + P, :])
                        x2s.append(xbt)
                    x2 = x2s[tt]
                    ps = psum.tile([P, 512], F32, tag="mm")
                    for kt in range(KD):
                        nc.tensor.matmul(
                            ps[:, :osz], aT[:, kt, t0:t0 + P],
                            wps[kt][:, :osz],
                            start=(kt == 0), stop=(kt == KD - 1),
                        )
                    nc.vector.tensor_add(out=x2[:, o0:o0 + osz],
                                         in0=ps[:, :osz],
                                         in1=x2[:, o0:o0 + osz])
            oacc = oacc_p.tile([P, NTT, D], F32)
            xn2T = xn2T_p.tile([P, KD, T], FR)
            ln2_tiles = []
            for tt, (t0, _) in enumerate(t_tiles_ov):
                x2 = x2s[tt]
                nc.vector.tensor_add(out=oacc[:, tt, :], in0=x2[:, :],
                                     in1=cf2_rep[:, :])
                ln2_tiles.append((x2, t0, 0, P))
            layernorm_transpose(ln2_tiles, xn2T)

            # ---------------- Phase D: MLP (hidden blocks in pairs) ----------------
            for hb2 in range(0, NHB, 2):
                hTs = []
                for hb in (hb2, hb2 + 1):
                    f1s = []
                    for kt in range(KD):
                        wt = w5_p.tile([P, 512], FR, tag="w")
                        nc.sync.dma_start(out=wt[:, :],
                                          in_=wf1_d[kt * P:(kt + 1) * P, hb * HB:(hb + 1) * HB])
                        f1s.append(wt)
                    hT = hT_p.tile([P, KH, T], FR)
                    for ht in range(KH):
                        for (n0, nsz) in n_blocks_T:
                            ps = psum.tile([P, 512], F32, tag="mm")
                            for kt in range(KD):
                                nc.tensor.matmul(
                                    ps[:, :nsz], f1s[kt][:, ht * P:(ht + 1) * P],
                                    xn2T[:, kt, n0:n0 + nsz],
                                    start=(kt == 0), stop=(kt == KD - 1),
                                )
                            nc.scalar.activation(
                                hT[:, ht, n0:n0 + nsz], ps[:, :nsz], func=AF.Gelu,
                                bias=cf1_sb[:, hb * KH + ht:hb * KH + ht + 1])
                    hTs.append(hT)
                for (o0, osz) in d_blocks:
                    f2s = []
                    for j, hb in enumerate((hb2, hb2 + 1)):
                        for kt in range(KH):
                            wt = w5_p.tile([P, 512], FR, tag="w")
                            nc.sync.dma_start(
                                out=wt[:, :osz],
                                in_=wf2_d[(hb * KH + kt) * P:(hb * KH + kt + 1) * P, o0:o0 + osz])
                            f2s.append(wt)
                    for tt, (t0, _) in enumerate(t_tiles_ov):
                        ps = psum.tile([P, 512], F32, tag="mm")
                        for j in range(2):
                            for kt in range(KH):
                                nc.tensor.matmul(
                                    ps[:, :osz], hTs[j][:, kt, t0:t0 + P],
                                    f2s[j * KH + kt][:, :osz],
                                    start=(j == 0 and kt == 0),
                                    stop=(j == 1 and kt == KH - 1),
                                )
                        nc.vector.tensor_add(out=oacc[:, tt, o0:o0 + osz],
                                             in0=oacc[:, tt, o0:o0 + osz],
                                             in1=ps[:, :osz])
            for tt, (t0, _) in enumerate(t_tiles_ov):
                f0 = fresh[tt]
                nc.sync.dma_start(out=out_d[g * T + t0 + f0:g * T + t0 + P, :],
                                  in_=oacc[f0:P, tt, :])

    nc.compile()
    return nc


def prepare_inputs(inputs, B, T, D, H, HID, n_cores):
    """Host-side folding/permutation. Returns per-core in_maps."""
    f8 = np.float64
    x = np.asarray(inputs["x"], np.float32)
    g1 = np.asarray(inputs["ln1_g"], f8)
    b1 = np.asarray(inputs["ln1_b"], f8)
    qkv_w = np.asarray(inputs["qkv_w"], f8)
    qkv_b = np.asarray(inputs["qkv_b"], f8)
    proj_w = np.asarray(inputs["proj_w"], np.float32)
    proj_b = np.asarray(inputs["proj_b"], f8)
    g2 = np.asarray(inputs["ln2_g"], f8)
    b2 = np.asarray(inputs["ln2_b"], f8)
    fc1_w = np.asarray(inputs["fc1_w"], f8)
    fc1_b = np.asarray(inputs["fc1_b"], f8)
    fc2_w = np.asarray(inputs["fc2_w"], np.float32)
    fc2_b = np.asarray(inputs["fc2_b"], f8)

    NPAIR = H // 2
    NHT = HID // P

    wq = (g1[:, None] * qkv_w).astype(np.float32)          # LN1 gamma folded
    cq = (b1 @ qkv_w + qkv_b).astype(np.float32)           # LN1 beta + qkv bias
    wq_, wk_, wv_ = wq[:, :D], wq[:, D:2 * D], wq[:, 2 * D:]
    wqkv = np.concatenate([
        wq_.reshape(D, NPAIR, P), wk_.reshape(D, NPAIR, P), wv_.reshape(D, NPAIR, P)
    ], axis=2).astype(np.float32)
    cq_, ck_, cv_ = cq[:D], cq[D:2 * D], cq[2 * D:]
    cq_t = np.stack([cq_.reshape(NPAIR, P), ck_.reshape(NPAIR, P),
                     cv_.reshape(NPAIR, P)], axis=1)       # [NPAIR, 3, P]
    cq_t = np.ascontiguousarray(cq_t.transpose(2, 0, 1).reshape(P, NPAIR * 3),
                                dtype=np.float32)

    wf1 = (g2[:, None] * fc1_w).astype(np.float32)
    cf1 = (b2 @ fc1_w + fc1_b).astype(np.float32)
    cf1_t = np.ascontiguousarray(cf1.reshape(NHT, P).T, dtype=np.float32)
    cf2 = fc2_b.astype(np.float32)
    xb = (np.asarray(inputs["x"], f8) + proj_b[None, None, :]).astype(np.float32)

    Bc = B // n_cores
    TOK = Bc * T
    shared = dict(wqkv=wqkv, cq=cq_t, wproj=np.ascontiguousarray(proj_w),
                  wfc1=wf1, cf1=cf1_t,
                  wfc2=np.ascontiguousarray(fc2_w), cf2=cf2)
    in_maps = []
    for c in range(n_cores):
        m = dict(shared)
        m["x"] = np.ascontiguousarray(x[c * Bc:(c + 1) * Bc].reshape(TOK, D))
        m["xb"] = np.ascontiguousarray(xb[c * Bc:(c + 1) * Bc].reshape(TOK, D))
        in_maps.append(m)
    return in_maps


_NC_CACHE = {}


def _get_nc(B, T, D, H, HD, HID):
    key = (B, T, D, H, HD, HID)
    if key not in _NC_CACHE:
        _NC_CACHE[key] = build_nc(B=B, T=T, D=D, H=H, HD=HD, HID=HID)
    return _NC_CACHE[key]


def _run(inputs, trace=False):
    from concourse.bass_utils import run_bass_kernel_spmd
    x = np.asarray(inputs["x"])
    B, T, D = x.shape
    H = 16
    HD = D // H
    HID = np.asarray(inputs["fc1_w"]).shape[1]
    n_cores = 8
    Bc = B // n_cores
    nc = _get_nc(Bc, T, D, H, HD, HID)
    in_maps = prepare_inputs(inputs, B, T, D, H, HID, n_cores)
    res = run_bass_kernel_spmd(nc, in_maps, list(range(n_cores)), trace=trace)
    out = np.concatenate(
        [res.results[c]["out"].reshape(Bc, T, D) for c in range(n_cores)], axis=0)
    return out, res


def kernel(**inputs) -> np.ndarray:
    out, _ = _run(inputs, trace=False)
    return out.astype(np.float32)



# revision 3
# speedup vs baseline: 2.2583x; 2.2583x over previous
"""Fused ViT transformer block on Trainium2, data-parallel over batch across 8 cores.

Per-core shard [4, 577, 1024] (D=1024, 16 heads, MLP 4096). Design notes:

  - All matmul operands are bf16 (1 PE cycle/row regardless of N/M, vs
    fp32r's 4x penalty at N<256 and fp32's 2x transposes). PSUM accumulation,
    the residual stream, and LN statistics stay f32; rel err ~1.5e-3.
  - Batches run in 2 groups of 2. Within a group every phase is weight-major,
    so each weight tile is DMA'd once per group as a single natural-slice
    descriptor from a host-side pre-permuted layout: ~98 DMAs total vs 895 in
    the fp32r baseline (each dma_start costs ~1.3us of shared HWDGE/DGE pipe).
  - Attention computes scores directly transposed, S^T = [k, q], with the key
    tile as the stationary operand, eliminating the baseline's 205k rows/batch
    of probability transposes. exp(S^T) goes to SBUF bf16 via the Act engine.
    AV uses V in [k, d] layout with an appended ones column so one PSUM
    accumulation chain yields both the head output [64, q] and the softmax
    denominators (row 64). A reciprocal row (DVE -> bf16) is broadcast to 64
    partitions by a K=1 PE matmul, Act-evicted to SBUF, and multiplied into
    the output on eviction (DVE quadrant-shifted write for the odd head).
  - Phase B is software-pipelined: AV/normalize of iteration n is emitted
    after QKV/scores of iteration n+1, so the PE does not wait on the Act
    exp stream. Groups' phases are interleaved (grp1's LN1 fills the PE gap
    at grp0's proj->LN2 boundary). wproj is prefetched during attention.
  - MLP hidden blocks run in pairs (8 k-tiles -> one PSUM accumulation per
    1024-col tile, single DVE eviction-add). fc1 bias+gelu fuse into the Act
    eviction. LN affines are folded into adjacent weights host-side; proj_b
    is pre-added to the residual x tiles on the idle GPSIMD engine and fc2_b
    folded in the same way.

Cost-model (TimelineSim): 1,136,051 ns/core (PE busy 981us = 86%);
measured on hw via paired differential bench: ~1.09 ms median
(fp32r baseline: 1,601,115 ns predicted / 2,595,817 ns measured).
"""

import numpy as np
from contextlib import ExitStack

import concourse.bass as bass
import concourse.mybir as mybir
import concourse.tile as tile
from concourse import bacc
from concourse.masks import make_identity

P = 128
F32 = mybir.dt.float32
BF = mybir.dt.bfloat16
AF = mybir.ActivationFunctionType
ALU = mybir.AluOpType


def build_nc(B=4, T=577, D=1024, H=16, HD=64, HID=4096, eps=1e-6):
    assert D % P == 0 and H % 2 == 0 and HD == 64 and HID % 512 == 0
    KD = D // P               # 8 k-tiles over model dim
    NPAIR = H // 2            # 8 head pairs
    NHB = HID // 512          # 8 hidden blocks
    KH = 512 // P             # 4 k-tiles per hidden block
    NHT = HID // P            # 32 fc1 output tiles
    GRP = 2                   # batches per weight-pass group
    NG = B // GRP
    scale = 1.0 / float(np.sqrt(HD))

    t_tiles = [(i, min(P, T - i)) for i in range(0, T, P)]      # exact tiles
    NTT = len(t_tiles)
    n_blocks = [(i, min(512, T - i)) for i in range(0, T, 512)]  # psum-bank cols

    nc = bacc.Bacc(None, target_bir_lowering=False, debug=False)

    x_d = nc.dram_tensor("x", [B * T, D], F32, kind="ExternalInput")
    wqkv_d = nc.dram_tensor("wqkv", [P, NPAIR, KD, 3 * P], BF, kind="ExternalInput")
    cq_d = nc.dram_tensor("cq", [P, NPAIR * 3], F32, kind="ExternalInput")
    wp_d = nc.dram_tensor("wproj", [P, KD, D], BF, kind="ExternalInput")
    cpb_d = nc.dram_tensor("cpb", [P, D], F32, kind="ExternalInput")
    wf1_d = nc.dram_tensor("wfc1", [P, NHB, KD, 512], BF, kind="ExternalInput")
    cf1_d = nc.dram_tensor("cf1", [P, NHT], F32, kind="ExternalInput")
    wf2_d = nc.dram_tensor("wfc2", [P, NHB, KH, D], BF, kind="ExternalInput")
    cf2_d = nc.dram_tensor("cf2", [P, D], F32, kind="ExternalInput")
    out_d = nc.dram_tensor("out", [B * T, D], F32, kind="ExternalOutput")

    with tile.TileContext(nc) as tc, ExitStack() as ctx:
        const = ctx.enter_context(tc.tile_pool(name="const", bufs=1))
        statp = ctx.enter_context(tc.tile_pool(name="stat", bufs=8))
        lnT_p = ctx.enter_context(tc.tile_pool(name="lnT", bufs=4))
        aT_p = ctx.enter_context(tc.tile_pool(name="aT", bufs=2))
        x2_p = ctx.enter_context(tc.tile_pool(name="x2", bufs=2))
        w_p = ctx.enter_context(tc.tile_pool(name="wpool", bufs=2))
        xin_p = ctx.enter_context(tc.tile_pool(name="xin", bufs=2))
        scr_p = ctx.enter_context(tc.tile_pool(name="scr", bufs=2))
        qkvt_p = ctx.enter_context(tc.tile_pool(name="qkvt", bufs=2))
        vkd_p = ctx.enter_context(tc.tile_pool(name="vkd", bufs=2))
        expst_p = ctx.enter_context(tc.tile_pool(name="expst", bufs=4))
        rinv_p = ctx.enter_context(tc.tile_pool(name="rinv", bufs=1))
        hT_p = ctx.enter_context(tc.tile_pool(name="hTp", bufs=2))
        pmm = ctx.enter_context(tc.tile_pool(name="pmm", bufs=2, space="PSUM"))
        pav = ctx.enter_context(tc.tile_pool(name="pav", bufs=2, space="PSUM"))

        ident = const.tile([P, P], F32)
        make_identity(nc, ident)
        identA = const.tile([P, P], BF)
        nc.vector.tensor_copy(out=identA[:, :], in_=ident[:, :])
        eps_t = const.tile([P, 1], F32)
        nc.vector.memset(eps_t, eps)
        ones_bf = const.tile([65, 64], BF)
        nc.vector.memset(ones_bf[64:65, :], 1.0)
        cq_sb = const.tile([P, NPAIR * 3], F32)
        nc.sync.dma_start(out=cq_sb[:, :], in_=cq_d[:, :])
        cf1_sb = const.tile([P, NHT], F32)
        nc.sync.dma_start(out=cf1_sb[:, :], in_=cf1_d[:, :])
        cpb_sb = const.tile([P, D], F32)
        nc.sync.dma_start(out=cpb_sb[:, :], in_=cpb_d[:, :])
        cf2_sb = const.tile([P, D], F32)
        nc.sync.dma_start(out=cf2_sb[:, :], in_=cf2_d[:, :])

        def ln_stats(src, tsz):
            """src: SBUF f32 AP [tsz, D]. Returns (mean, istd) stat tiles."""
            stats = statp.tile([P, 2, 6], F32, tag="bnst")
            xv = src.rearrange("p (s f) -> p s f", s=2)
            for s in range(2):
                nc.vector.bn_stats(out=stats[0:tsz, s, :], in_=xv[:, s, :])
            mv = statp.tile([P, 2], F32, tag="mv")
            nc.vector.bn_aggr(out=mv[0:tsz, :], in_=stats[0:tsz])
            istd = statp.tile([P, 1], F32, tag="istd")
            nc.scalar.activation(out=istd[0:tsz], in_=mv[0:tsz, 1:2],
                                 func=AF.Sqrt, bias=eps_t[0:tsz])
            nc.vector.reciprocal(out=istd[0:tsz], in_=istd[0:tsz])
            return mv, istd

        def ln_norm_tr(src, tsz, negmi, istd, dstT, t0):
            """Normalize src with (mv, istd) -> bf16, PE-transpose into
            dstT[:, :, t0:t0+tsz]."""
            scr = scr_p.tile([P, D], BF)
            nc.vector.tensor_scalar(
                out=scr[0:tsz, :], in0=src,
                scalar1=negmi[0:tsz, 0:1], scalar2=istd[0:tsz],
                op0=ALU.subtract, op1=ALU.mult,
            )
            for kt0 in range(0, KD, 4):
                trp = pav.tile([P, 4, P], BF, tag="av")
                for j4 in range(4):
                    kt = kt0 + j4
                    nc.tensor.matmul(
                        trp[:, j4, 0:tsz], scr[0:tsz, kt * P:(kt + 1) * P],
                        identA[0:tsz, 0:tsz], is_transpose=True,
                        start=(j4 == 0), stop=(j4 == 3),
                    )
                nc.vector.tensor_copy(
                    out=dstT[:, kt0:kt0 + 4, t0:t0 + tsz],
                    in_=trp[:, :, 0:tsz])

        def ln_transpose(src, tsz, dstT, t0):
            negmi, istd = ln_stats(src, tsz)
            ln_norm_tr(src, tsz, negmi, istd, dstT, t0)

        def phase_a(grp):
            """LN1 for both batches of the group -> per-gg xn1T tiles."""
            xn1Ts = []
            for gg in range(GRP):
                g = grp * GRP + gg
                xn1T = lnT_p.tile([P, KD, T], BF, tag="lnT", name=f"xn1T{gg}")
                xn1Ts.append(xn1T)
                for (t0, tsz) in t_tiles:
                    xt = xin_p.tile([P, D], F32)
                    nc.sync.dma_start(out=xt[0:tsz, :],
                                      in_=x_d[g * T + t0:g * T + t0 + tsz, :])
                    ln_transpose(xt[0:tsz, :], tsz, xn1T, t0)
            return xn1Ts

        def attn_stage1(p, gg, wq, xn1T):
            """QKV matmuls + evicts, scores+exp for both heads, V->[k,d]."""
            qT = qkvt_p.tile([P, T], BF, tag="tq")
            kTt = qkvt_p.tile([P, T], BF, tag="tk")
            vT = qkvt_p.tile([P, T], BF, tag="tv")
            for i, dst in enumerate((qT, kTt, vT)):
                ps = pmm.tile([P, T], F32, tag="mm")
                for kt in range(KD):
                    lhsT = wq[:, kt, i * P:(i + 1) * P]
                    for (n0, nsz) in n_blocks:
                        nc.tensor.matmul(
                            ps[:, n0:n0 + nsz], lhsT,
                            xn1T[:, kt, n0:n0 + nsz],
                            start=(kt == 0), stop=(kt == KD - 1),
                        )
                nc.vector.tensor_scalar(
                    out=dst[:, :], in0=ps[:, 0:T],
                    scalar1=cq_sb[:, p * 3 + i:p * 3 + i + 1],
                    scalar2=None, op0=ALU.add)
            expsts = []
            for hi in range(2):
                hb0 = 64 * hi
                expst = expst_p.tile([P, NTT, T], BF, tag="expst")
                for kti, (k0, ksz) in enumerate(t_tiles):
                    st = pmm.tile([P, T], F32, tag="mm")
                    lhsT = kTt[hb0:hb0 + 64, k0:k0 + ksz]
                    for (n0, nsz) in n_blocks:
                        nc.tensor.matmul(st[0:ksz, n0:n0 + nsz], lhsT,
                                         qT[hb0:hb0 + 64, n0:n0 + nsz])
                    nc.scalar.activation(out=expst[0:ksz, kti, :],
                                         in_=st[0:ksz, 0:T],
                                         func=AF.Exp, scale=scale)
                expsts.append(expst)
            vkd2 = vkd_p.tile([P, NTT, 130], BF)
            nc.vector.memset(vkd2[:, :, 64:65], 1.0)
            nc.vector.memset(vkd2[:, :, 129:130], 1.0)
            for kti, (k0, ksz) in enumerate(t_tiles):
                trp = pav.tile([P, P], BF, tag="av")
                nc.tensor.matmul(trp[0:ksz, :], vT[:, k0:k0 + ksz],
                                 identA[:, :], is_transpose=True)
                dstv = vkd2[0:ksz, kti, 0:130].rearrange(
                    "p (s f) -> p s f", s=2)[:, :, 0:64]
                nc.vector.tensor_copy(
                    out=dstv,
                    in_=trp[0:ksz, :].rearrange("p (s f) -> p s f", s=2))
            return (p, gg, expsts, vkd2)

        def attn_stage2(state, aTs):
            """AV with fused denominator row; normalize on eviction."""
            p, gg, expsts, vkd2 = state
            muls = []
            for hi in range(2):
                av = pav.tile([65, T], F32, tag="av")
                for kti, (k0, ksz) in enumerate(t_tiles):
                    lhsT = vkd2[0:ksz, kti, hi * 65:(hi + 1) * 65]
                    for (n0, nsz) in n_blocks:
                        nc.tensor.matmul(
                            av[:, n0:n0 + nsz], lhsT,
                            expsts[hi][0:ksz, kti, n0:n0 + nsz],
                            start=(kti == 0), stop=(kti == NTT - 1),
                        )
                rin = rinv_p.tile([65, T], BF, tag="rin")
                with nc.allow_low_precision(reason="softmax denom bf16"):
                    nc.vector.reciprocal(out=rin[64:65, :],
                                         in_=av[64:65, 0:T])
                bc = pmm.tile([64, T], F32, tag="mm")
                for (n0, nsz) in n_blocks:
                    nc.tensor.matmul(bc[:, n0:n0 + nsz],
                                     ones_bf[64:65, :],
                                     rin[64:65, n0:n0 + nsz])
                bcs = rinv_p.tile([64, T], BF, tag="bcs")
                nc.scalar.copy(out=bcs[:, :], in_=bc[:, 0:T])
                muls.append((av, bcs))
            for hi, (av, bcs) in enumerate(muls):
                hb0 = 64 * hi
                nc.vector.tensor_mul(out=aTs[gg][hb0:hb0 + 64, p, :],
                                     in0=av[0:64, 0:T], in1=bcs[:, 0:T])

        def phase_b(grp, xn1Ts):
            """Attention, software-pipelined: stage2 of iteration n is
            emitted after stage1 of iteration n+1 so the PE never waits
            on the Act exp stream."""
            aTs = [aT_p.tile([P, KD, T], BF, tag="aT", name=f"aT{gg}")
                   for gg in range(GRP)]
            wp = None
            pending = None
            for p in range(NPAIR):
                if p == NPAIR - 1:
                    # prefetch wproj so phase C starts without a DMA stall
                    wp = w_p.tile([P, KD, D], BF, tag="W")
                    nc.sync.dma_start(out=wp[:, :, :], in_=wp_d[:, :, :])
                wq = w_p.tile([P, KD, 3 * P], BF, tag="W")
                nc.sync.dma_start(out=wq[:, :, :], in_=wqkv_d[:, p, :, :])
                for gg in range(GRP):
                    st1 = attn_stage1(p, gg, wq, xn1Ts[gg])
                    if pending is not None:
                        attn_stage2(pending, aTs)
                    pending = st1
            attn_stage2(pending, aTs)
            return aTs, wp

        def phase_c(grp, aTs, wp):
            xn2Ts = []
            x2s_t = []
            for gg in range(GRP):
                g = grp * GRP + gg
                xn2T = lnT_p.tile([P, KD, T], BF, tag="lnT", name=f"xn2T{gg}")
                xn2Ts.append(xn2T)
                x2 = x2_p.tile([P, NTT, D], F32, tag="x2", name=f"x2_{gg}")
                x2s_t.append(x2)
                stats2 = []
                for j, (t0, tsz) in enumerate(t_tiles):
                    xt = xin_p.tile([P, D], F32)
                    nc.sync.dma_start(out=xt[0:tsz, :],
                                      in_=x_d[g * T + t0:g * T + t0 + tsz, :])
                    nc.gpsimd.tensor_add(out=xt[0:tsz, :], in0=xt[0:tsz, :],
                                         in1=cpb_sb[0:tsz, :])
                    ps = pmm.tile([P, D], F32, tag="mm")
                    for kt in range(KD):
                        lhsT = aTs[gg][:, kt, t0:t0 + tsz]
                        for o in range(2):
                            nc.tensor.matmul(
                                ps[0:tsz, o * 512:(o + 1) * 512], lhsT,
                                wp[:, kt, o * 512:(o + 1) * 512],
                                start=(kt == 0), stop=(kt == KD - 1),
                            )
                    x2s = x2[0:tsz, j, :]
                    nc.vector.tensor_add(out=x2s, in0=ps[0:tsz, :], in1=xt[0:tsz, :])
                    stats2.append(ln_stats(x2s, tsz))
                for j, (t0, tsz) in enumerate(t_tiles):
                    x2s = x2[0:tsz, j, :]
                    mv, istd = stats2[j]
                    ln_norm_tr(x2s, tsz, mv, istd, xn2T, t0)
                    nc.gpsimd.tensor_add(out=x2s, in0=x2s, in1=cf2_sb[0:tsz, :])
            return xn2Ts, x2s_t

        def phase_d(grp, xn2Ts, x2s_t):
            for hbp in range(NHB // 2):
                f1 = w_p.tile([P, 2, KD, 512], BF, tag="W")
                nc.sync.dma_start(out=f1[:, :, :, :],
                                  in_=wf1_d[:, 2 * hbp:2 * hbp + 2, :, :])
                f2 = w_p.tile([P, 2, KH, D], BF, tag="W")
                nc.sync.dma_start(out=f2[:, :, :, :],
                                  in_=wf2_d[:, 2 * hbp:2 * hbp + 2, :, :])
                hTs = []
                for gg in range(GRP):
                    hT = hT_p.tile([P, 2 * KH, T], BF)
                    for hb2 in range(2):
                        for ht in range(KH):
                            ps = pmm.tile([P, T], F32, tag="mm")
                            for kt in range(KD):
                                lhsT = f1[:, hb2, kt, ht * P:(ht + 1) * P]
                                for (n0, nsz) in n_blocks:
                                    nc.tensor.matmul(
                                        ps[:, n0:n0 + nsz], lhsT,
                                        xn2Ts[gg][:, kt, n0:n0 + nsz],
                                        start=(kt == 0), stop=(kt == KD - 1),
                                    )
                            hidx = (2 * hbp + hb2) * KH + ht
                            nc.scalar.activation(
                                out=hT[:, hb2 * KH + ht, :], in_=ps[:, 0:T],
                                func=AF.Gelu, bias=cf1_sb[:, hidx:hidx + 1])
                    hTs.append(hT)
                for gg in range(GRP):
                    hT = hTs[gg]
                    for j, (t0, tsz) in enumerate(t_tiles):
                        ps = pmm.tile([P, D], F32, tag="mm")
                        for k8 in range(2 * KH):
                            lhsT = hT[:, k8, t0:t0 + tsz]
                            for o in range(2):
                                nc.tensor.matmul(
                                    ps[0:tsz, o * 512:(o + 1) * 512], lhsT,
                                    f2[:, k8 // KH, k8 % KH, o * 512:(o + 1) * 512],
                                    start=(k8 == 0), stop=(k8 == 2 * KH - 1),
                                )
                        x2s = x2s_t[gg][0:tsz, j, :]
                        nc.vector.tensor_add(out=x2s, in0=x2s, in1=ps[0:tsz, :])
            for gg in range(GRP):
                g = grp * GRP + gg
                for j, (t0, tsz) in enumerate(t_tiles):
                    nc.sync.dma_start(out=out_d[g * T + t0:g * T + t0 + tsz, :],
                                      in_=x2s_t[gg][0:tsz, j, :])

        # Interleaved emission: grp1's LN1 fills the PE gap at grp0's
        # C->D boundary (proj-evict -> stats -> norm chain on DVE).
        a0 = phase_a(0)
        aT0, wp0 = phase_b(0, a0)
        c0 = phase_c(0, aT0, wp0)
        a1 = phase_a(1)
        phase_d(0, *c0)
        aT1, wp1 = phase_b(1, a1)
        c1 = phase_c(1, aT1, wp1)
        phase_d(1, *c1)

    nc.compile()
    return nc


def prepare_inputs(inputs, B, T, D, H, HID, n_cores):
    """Host-side folding + weight permutation into DMA-friendly layouts."""
    f8 = np.float64
    bf16 = mybir.dt.np(BF)
    x = np.asarray(inputs["x"], np.float32)
    g1 = np.asarray(inputs["ln1_g"], f8)
    b1 = np.asarray(inputs["ln1_b"], f8)
    qkv_w = np.asarray(inputs["qkv_w"], f8)
    qkv_b = np.asarray(inputs["qkv_b"], f8)
    proj_w = np.asarray(inputs["proj_w"], np.float32)
    proj_b = np.asarray(inputs["proj_b"], np.float32)
    g2 = np.asarray(inputs["ln2_g"], f8)
    b2 = np.asarray(inputs["ln2_b"], f8)
    fc1_w = np.asarray(inputs["fc1_w"], f8)
    fc1_b = np.asarray(inputs["fc1_b"], f8)
    fc2_w = np.asarray(inputs["fc2_w"], np.float32)
    fc2_b = np.asarray(inputs["fc2_b"], np.float32)

    KD = D // P
    NPAIR = H // 2
    NHB = HID // 512
    KH = 512 // P
    NHT = HID // P

    wq = (g1[:, None] * qkv_w).astype(np.float32)
    cq = (b1 @ qkv_w + qkv_b).astype(np.float32)
    wq_, wk_, wv_ = wq[:, :D], wq[:, D:2 * D], wq[:, 2 * D:]
    Wq = np.stack([
        np.concatenate([wq_[:, p * P:(p + 1) * P], wk_[:, p * P:(p + 1) * P],
                        wv_[:, p * P:(p + 1) * P]], axis=1)
        for p in range(NPAIR)
    ], axis=0)                                            # [NPAIR, D, 384]
    wqkv_h = np.ascontiguousarray(
        Wq.reshape(NPAIR, KD, P, 3 * P).transpose(2, 0, 1, 3)).astype(bf16)
    cq_, ck_, cv_ = cq[:D], cq[D:2 * D], cq[2 * D:]
    cq_t = np.stack([cq_.reshape(NPAIR, P), ck_.reshape(NPAIR, P),
                     cv_.reshape(NPAIR, P)], axis=1)      # [NPAIR, 3, P]
    cq_t = np.ascontiguousarray(cq_t.transpose(2, 0, 1).reshape(P, NPAIR * 3),
                                dtype=np.float32)

    wproj_h = np.ascontiguousarray(
        proj_w.reshape(KD, P, D).transpose(1, 0, 2)).astype(bf16)
    wf1 = (g2[:, None] * fc1_w).astype(np.float32)
    cf1 = (b2 @ fc1_w + fc1_b).astype(np.float32)
    wf1_h = np.ascontiguousarray(
        wf1.reshape(KD, P, NHB, 512).transpose(1, 2, 0, 3)).astype(bf16)
    cf1_t = np.ascontiguousarray(cf1.reshape(NHT, P).T, dtype=np.float32)
    wf2_h = np.ascontiguousarray(
        fc2_w.reshape(NHB, KH, P, D).transpose(2, 0, 1, 3)).astype(bf16)
    cpb_h = np.ascontiguousarray(np.broadcast_to(proj_b, (P, D)), np.float32)
    cf2_h = np.ascontiguousarray(np.broadcast_to(fc2_b, (P, D)), np.float32)

    Bc = B // n_cores
    TOK = Bc * T
    shared = dict(wqkv=wqkv_h, cq=cq_t, wproj=wproj_h, cpb=cpb_h,
                  wfc1=wf1_h, cf1=cf1_t, wfc2=wf2_h, cf2=cf2_h)
    in_maps = []
    for c in range(n_cores):
        m = dict(shared)
        m["x"] = np.ascontiguousarray(x[c * Bc:(c + 1) * Bc].reshape(TOK, D))
        in_maps.append(m)
    return in_maps


_NC_CACHE = {}


def _get_nc(B, T, D, H, HD, HID):
    key = (B, T, D, H, HD, HID)
    if key not in _NC_CACHE:
        _NC_CACHE[key] = build_nc(B=B, T=T, D=D, H=H, HD=HD, HID=HID)
    return _NC_CACHE[key]


def _run(inputs, trace=False):
    from concourse.bass_utils import run_bass_kernel_spmd
    x = np.asarray(inputs["x"])
    B, T, D = x.shape
    H = 16
    HD = D // H
    HID = np.asarray(inputs["fc1_w"]).shape[1]
    n_cores = 8
    Bc = B // n_cores
    nc = _get_nc(Bc, T, D, H, HD, HID)
    in_maps = prepare_inputs(inputs, B, T, D, H, HID, n_cores)
    res = run_bass_kernel_spmd(nc, in_maps, list(range(n_cores)), trace=trace)
    out = np.concatenate(
        [res.results[c]["out"].reshape(Bc, T, D) for c in range(n_cores)], axis=0)
    return out, res


def kernel(**inputs) -> np.ndarray:
    out, _ = _run(inputs, trace=False)
    return out.astype(np.float32)


# revision 4
# speedup vs baseline: 2.8340x; 1.2549x over previous
"""Fused ViT transformer block on Trainium2, data-parallel over batch across 8 cores.

Per-core shard [4, 577, 1024] (D=1024, 16 heads, MLP 4096). Design notes:

  - All matmul operands are bf16 (1 PE cycle/row regardless of N/M, vs
    fp32r's 4x penalty at N<256 and fp32's 2x transposes). PSUM accumulation,
    the residual stream, and LN statistics stay f32; rel err ~1.5e-3.
  - Batches run in 2 groups of 2. Within a group every phase is weight-major,
    so each weight tile is DMA'd once per group as a single natural-slice
    descriptor from a host-side pre-permuted layout: ~98 DMAs total vs 895 in
    the fp32r baseline (each dma_start costs ~1.3us of shared HWDGE/DGE pipe).
  - Attention computes scores directly transposed, S^T = [k, q], with the key
    tile as the stationary operand, eliminating the baseline's 205k rows/batch
    of probability transposes. exp(S^T) goes to SBUF bf16 via the Act engine.
    AV uses V in [k, d] layout with an appended ones column so one PSUM
    accumulation chain yields both the head output [64, q] and the softmax
    denominators (row 64). A reciprocal row (DVE -> bf16) is broadcast to 64
    partitions by a K=1 PE matmul, Act-evicted to SBUF, and multiplied into
    the output on eviction (DVE quadrant-shifted write for the odd head).
  - Phase B is software-pipelined: AV/normalize of iteration n is emitted
    after QKV/scores of iteration n+1, so the PE does not wait on the Act
    exp stream. Groups' phases are interleaved (grp1's LN1 fills the PE gap
    at grp0's proj->LN2 boundary). wproj is prefetched during attention.
  - MLP hidden blocks run in pairs (8 k-tiles -> one PSUM accumulation per
    1024-col tile, single DVE eviction-add). fc1 bias+gelu fuse into the Act
    eviction. LN affines are folded into adjacent weights host-side; proj_b
    is pre-added to the residual x tiles on the idle GPSIMD engine and fc2_b
    folded in the same way.

Cost-model (TimelineSim): 1,129,313 ns/core (PE busy 981us = 87%);
measured on hw via paired differential bench: ~1.09 ms median
(fp32r baseline: 1,601,115 ns predicted / 2,595,817 ns measured).
"""

import numpy as np
from contextlib import ExitStack

import concourse.bass as bass
import concourse.mybir as mybir
import concourse.tile as tile
from concourse import bacc
from concourse.masks import make_identity

P = 128
F32 = mybir.dt.float32
BF = mybir.dt.bfloat16
AF = mybir.ActivationFunctionType
ALU = mybir.AluOpType


def build_nc(B=4, T=577, D=1024, H=16, HD=64, HID=4096, eps=1e-6):
    assert D % P == 0 and H % 2 == 0 and HD == 64 and HID % 512 == 0
    KD = D // P               # 8 k-tiles over model dim
    NPAIR = H // 2            # 8 head pairs
    NHB = HID // 512          # 8 hidden blocks
    KH = 512 // P             # 4 k-tiles per hidden block
    NHT = HID // P            # 32 fc1 output tiles
    GRP = 2                   # batches per weight-pass group
    NG = B // GRP
    scale = 1.0 / float(np.sqrt(HD))

    t_tiles = [(i, min(P, T - i)) for i in range(0, T, P)]      # exact tiles
    NTT = len(t_tiles)
    n_blocks = [(i, min(512, T - i)) for i in range(0, T, 512)]  # psum-bank cols

    nc = bacc.Bacc(None, target_bir_lowering=False, debug=False)

    x_d = nc.dram_tensor("x", [B * T, D], F32, kind="ExternalInput")
    wqkv_d = nc.dram_tensor("wqkv", [P, NPAIR, KD, 3 * P], BF, kind="ExternalInput")
    cq_d = nc.dram_tensor("cq", [P, NPAIR * 3], F32, kind="ExternalInput")
    wp_d = nc.dram_tensor("wproj", [P, KD, D], BF, kind="ExternalInput")
    cpb_d = nc.dram_tensor("cpb", [P, D], F32, kind="ExternalInput")
    wf1_d = nc.dram_tensor("wfc1", [P, NHB, KD, 512], BF, kind="ExternalInput")
    cf1_d = nc.dram_tensor("cf1", [P, NHT], F32, kind="ExternalInput")
    wf2_d = nc.dram_tensor("wfc2", [P, NHB, KH, D], BF, kind="ExternalInput")
    cf2_d = nc.dram_tensor("cf2", [P, D], F32, kind="ExternalInput")
    out_d = nc.dram_tensor("out", [B * T, D], F32, kind="ExternalOutput")

    with tile.TileContext(nc) as tc, ExitStack() as ctx:
        const = ctx.enter_context(tc.tile_pool(name="const", bufs=1))
        statp = ctx.enter_context(tc.tile_pool(name="stat", bufs=8))
        lnT_p = ctx.enter_context(tc.tile_pool(name="lnT", bufs=4))
        aT_p = ctx.enter_context(tc.tile_pool(name="aT", bufs=2))
        x2_p = ctx.enter_context(tc.tile_pool(name="x2", bufs=2))
        w_p = ctx.enter_context(tc.tile_pool(name="wpool", bufs=2))
        xin_p = ctx.enter_context(tc.tile_pool(name="xin", bufs=2))
        scr_p = ctx.enter_context(tc.tile_pool(name="scr", bufs=3))
        qkvt_p = ctx.enter_context(tc.tile_pool(name="qkvt", bufs=3))
        vkd_p = ctx.enter_context(tc.tile_pool(name="vkd", bufs=3))
        expst_p = ctx.enter_context(tc.tile_pool(name="expst", bufs=4))
        rinv_p = ctx.enter_context(tc.tile_pool(name="rinv", bufs=1))
        hT_p = ctx.enter_context(tc.tile_pool(name="hTp", bufs=2))
        pmm = ctx.enter_context(tc.tile_pool(name="pmm", bufs=2, space="PSUM"))
        pav = ctx.enter_context(tc.tile_pool(name="pav", bufs=2, space="PSUM"))

        ident = const.tile([P, P], F32)
        make_identity(nc, ident)
        identA = const.tile([P, P], BF)
        nc.vector.tensor_copy(out=identA[:, :], in_=ident[:, :])
        eps_t = const.tile([P, 1], F32)
        nc.vector.memset(eps_t, eps)
        ones_bf = const.tile([65, 64], BF)
        nc.vector.memset(ones_bf[64:65, :], 1.0)
        cq_sb = const.tile([P, NPAIR * 3], F32)
        nc.sync.dma_start(out=cq_sb[:, :], in_=cq_d[:, :])
        cf1_sb = const.tile([P, NHT], F32)
        nc.sync.dma_start(out=cf1_sb[:, :], in_=cf1_d[:, :])
        cpb_sb = const.tile([P, D], F32)
        nc.sync.dma_start(out=cpb_sb[:, :], in_=cpb_d[:, :])
        cf2_sb = const.tile([P, D], F32)
        nc.sync.dma_start(out=cf2_sb[:, :], in_=cf2_d[:, :])

        def ln_stats(src, tsz):
            """src: SBUF f32 AP [tsz, D]. Returns (mean, istd) stat tiles."""
            stats = statp.tile([P, 2, 6], F32, tag="bnst")
            xv = src.rearrange("p (s f) -> p s f", s=2)
            for s in range(2):
                nc.vector.bn_stats(out=stats[0:tsz, s, :], in_=xv[:, s, :])
            mv = statp.tile([P, 2], F32, tag="mv")
            nc.vector.bn_aggr(out=mv[0:tsz, :], in_=stats[0:tsz])
            istd = statp.tile([P, 1], F32, tag="istd")
            nc.scalar.activation(out=istd[0:tsz], in_=mv[0:tsz, 1:2],
                                 func=AF.Sqrt, bias=eps_t[0:tsz])
            nc.vector.reciprocal(out=istd[0:tsz], in_=istd[0:tsz])
            return mv, istd

        def ln_norm_tr(src, tsz, negmi, istd, dstT, t0):
            """Normalize src with (mv, istd) -> bf16, PE-transpose into
            dstT[:, :, t0:t0+tsz]."""
            scr = scr_p.tile([P, D], BF)
            nc.vector.tensor_scalar(
                out=scr[0:tsz, :], in0=src,
                scalar1=negmi[0:tsz, 0:1], scalar2=istd[0:tsz],
                op0=ALU.subtract, op1=ALU.mult,
            )
            for kt0 in range(0, KD, 4):
                trp = pav.tile([P, 4, P], BF, tag="av")
                for j4 in range(4):
                    kt = kt0 + j4
                    nc.tensor.matmul(
                        trp[:, j4, 0:tsz], scr[0:tsz, kt * P:(kt + 1) * P],
                        identA[0:tsz, 0:tsz], is_transpose=True,
                        start=(j4 == 0), stop=(j4 == 3),
                    )
                nc.vector.tensor_copy(
                    out=dstT[:, kt0:kt0 + 4, t0:t0 + tsz],
                    in_=trp[:, :, 0:tsz])

        def ln_transpose(src, tsz, dstT, t0):
            negmi, istd = ln_stats(src, tsz)
            ln_norm_tr(src, tsz, negmi, istd, dstT, t0)

        def phase_a(grp):
            """LN1 for both batches of the group -> per-gg xn1T tiles."""
            xn1Ts = []
            for gg in range(GRP):
                g = grp * GRP + gg
                xn1T = lnT_p.tile([P, KD, T], BF, tag="lnT", name=f"xn1T{gg}")
                xn1Ts.append(xn1T)
                for (t0, tsz) in t_tiles:
                    xt = xin_p.tile([P, D], F32)
                    nc.sync.dma_start(out=xt[0:tsz, :],
                                      in_=x_d[g * T + t0:g * T + t0 + tsz, :])
                    ln_transpose(xt[0:tsz, :], tsz, xn1T, t0)
            return xn1Ts

        def attn_stage1(p, gg, wq, xn1T):
            """QKV matmuls + evicts, scores+exp for both heads, V->[k,d]."""
            qT = qkvt_p.tile([P, T], BF, tag="tq")
            kTt = qkvt_p.tile([P, T], BF, tag="tk")
            vT = qkvt_p.tile([P, T], BF, tag="tv")
            for i, dst in enumerate((qT, kTt, vT)):
                ps = pmm.tile([P, T], F32, tag="mm")
                for kt in range(KD):
                    lhsT = wq[:, kt, i * P:(i + 1) * P]
                    for (n0, nsz) in n_blocks:
                        nc.tensor.matmul(
                            ps[:, n0:n0 + nsz], lhsT,
                            xn1T[:, kt, n0:n0 + nsz],
                            start=(kt == 0), stop=(kt == KD - 1),
                        )
                nc.vector.tensor_scalar(
                    out=dst[:, :], in0=ps[:, 0:T],
                    scalar1=cq_sb[:, p * 3 + i:p * 3 + i + 1],
                    scalar2=None, op0=ALU.add)
            expsts = []
            for hi in range(2):
                hb0 = 64 * hi
                expst = expst_p.tile([P, NTT, T], BF, tag="expst")
                for kti, (k0, ksz) in enumerate(t_tiles):
                    st = pmm.tile([P, T], F32, tag="mm")
                    lhsT = kTt[hb0:hb0 + 64, k0:k0 + ksz]
                    for (n0, nsz) in n_blocks:
                        nc.tensor.matmul(st[0:ksz, n0:n0 + nsz], lhsT,
                                         qT[hb0:hb0 + 64, n0:n0 + nsz])
                    nc.scalar.activation(out=expst[0:ksz, kti, :],
                                         in_=st[0:ksz, 0:T],
                                         func=AF.Exp, scale=scale)
                expsts.append(expst)
            vkd2 = vkd_p.tile([P, NTT, 130], BF)
            nc.vector.memset(vkd2[:, :, 64:65], 1.0)
            nc.vector.memset(vkd2[:, :, 129:130], 1.0)
            for kti, (k0, ksz) in enumerate(t_tiles):
                trp = pav.tile([P, P], BF, tag="av")
                nc.tensor.matmul(trp[0:ksz, :], vT[:, k0:k0 + ksz],
                                 identA[:, :], is_transpose=True)
                dstv = vkd2[0:ksz, kti, 0:130].rearrange(
                    "p (s f) -> p s f", s=2)[:, :, 0:64]
                nc.vector.tensor_copy(
                    out=dstv,
                    in_=trp[0:ksz, :].rearrange("p (s f) -> p s f", s=2))
            return (p, gg, expsts, vkd2)

        def attn_stage2(state, aTs):
            """AV with fused denominator row; normalize on eviction."""
            p, gg, expsts, vkd2 = state
            muls = []
            for hi in range(2):
                av = pav.tile([65, T], F32, tag="av")
                for kti, (k0, ksz) in enumerate(t_tiles):
                    lhsT = vkd2[0:ksz, kti, hi * 65:(hi + 1) * 65]
                    for (n0, nsz) in n_blocks:
                        nc.tensor.matmul(
                            av[:, n0:n0 + nsz], lhsT,
                            expsts[hi][0:ksz, kti, n0:n0 + nsz],
                            start=(kti == 0), stop=(kti == NTT - 1),
                        )
                rin = rinv_p.tile([65, T], BF, tag="rin")
                with nc.allow_low_precision(reason="softmax denom bf16"):
                    nc.vector.reciprocal(out=rin[64:65, :],
                                         in_=av[64:65, 0:T])
                bc = pmm.tile([64, T], F32, tag="mm")
                for (n0, nsz) in n_blocks:
                    nc.tensor.matmul(bc[:, n0:n0 + nsz],
                                     ones_bf[64:65, :],
                                     rin[64:65, n0:n0 + nsz])
                bcs = rinv_p.tile([64, T], BF, tag="bcs")
                nc.scalar.copy(out=bcs[:, :], in_=bc[:, 0:T])
                muls.append((av, bcs))
            for hi, (av, bcs) in enumerate(muls):
                hb0 = 64 * hi
                nc.vector.tensor_mul(out=aTs[gg][hb0:hb0 + 64, p, :],
                                     in0=av[0:64, 0:T], in1=bcs[:, 0:T])

        def phase_b(grp, xn1Ts):
            """Attention, software-pipelined: stage2 of iteration n is
            emitted after stage1 of iteration n+1 so the PE never waits
            on the Act exp stream."""
            aTs = [aT_p.tile([P, KD, T], BF, tag="aT", name=f"aT{gg}")
                   for gg in range(GRP)]
            wp = None
            pending = None
            for p in range(NPAIR):
                if p == NPAIR - 1:
                    # prefetch wproj so phase C starts without a DMA stall
                    wp = w_p.tile([P, KD, D], BF, tag="W")
                    nc.sync.dma_start(out=wp[:, :, :], in_=wp_d[:, :, :])
                wq = w_p.tile([P, KD, 3 * P], BF, tag="W")
                nc.sync.dma_start(out=wq[:, :, :], in_=wqkv_d[:, p, :, :])
                for gg in range(GRP):
                    st1 = attn_stage1(p, gg, wq, xn1Ts[gg])
                    if pending is not None:
                        attn_stage2(pending, aTs)
                    pending = st1
            attn_stage2(pending, aTs)
            return aTs, wp

        def phase_c(grp, aTs, wp):
            xn2Ts = []
            x2s_t = []
            for gg in range(GRP):
                g = grp * GRP + gg
                xn2T = lnT_p.tile([P, KD, T], BF, tag="lnT", name=f"xn2T{gg}")
                xn2Ts.append(xn2T)
                x2 = x2_p.tile([P, NTT, D], F32, tag="x2", name=f"x2_{gg}")
                x2s_t.append(x2)
                stats2 = []
                for j, (t0, tsz) in enumerate(t_tiles):
                    xt = xin_p.tile([P, D], F32)
                    nc.sync.dma_start(out=xt[0:tsz, :],
                                      in_=x_d[g * T + t0:g * T + t0 + tsz, :])
                    nc.gpsimd.tensor_add(out=xt[0:tsz, :], in0=xt[0:tsz, :],
                                         in1=cpb_sb[0:tsz, :])
                    ps = pmm.tile([P, D], F32, tag="mm")
                    for kt in range(KD):
                        lhsT = aTs[gg][:, kt, t0:t0 + tsz]
                        for o in range(2):
                            nc.tensor.matmul(
                                ps[0:tsz, o * 512:(o + 1) * 512], lhsT,
                                wp[:, kt, o * 512:(o + 1) * 512],
                                start=(kt == 0), stop=(kt == KD - 1),
                            )
                    x2s = x2[0:tsz, j, :]
                    nc.vector.tensor_add(out=x2s, in0=ps[0:tsz, :], in1=xt[0:tsz, :])
                    stats2.append(ln_stats(x2s, tsz))
                for j, (t0, tsz) in enumerate(t_tiles):
                    x2s = x2[0:tsz, j, :]
                    mv, istd = stats2[j]
                    ln_norm_tr(x2s, tsz, mv, istd, xn2T, t0)
                    nc.gpsimd.tensor_add(out=x2s, in0=x2s, in1=cf2_sb[0:tsz, :])
            return xn2Ts, x2s_t

        def phase_d(grp, xn2Ts, x2s_t):
            for hbp in range(NHB // 2):
                f1 = w_p.tile([P, 2, KD, 512], BF, tag="W")
                nc.sync.dma_start(out=f1[:, :, :, :],
                                  in_=wf1_d[:, 2 * hbp:2 * hbp + 2, :, :])
                f2 = w_p.tile([P, 2, KH, D], BF, tag="W")
                nc.sync.dma_start(out=f2[:, :, :, :],
                                  in_=wf2_d[:, 2 * hbp:2 * hbp + 2, :, :])
                hTs = []
                for gg in range(GRP):
                    hT = hT_p.tile([P, 2 * KH, T], BF)
                    for hb2 in range(2):
                        for ht in range(KH):
                            ps = pmm.tile([P, T], F32, tag="mm")
                            for kt in range(KD):
                                lhsT = f1[:, hb2, kt, ht * P:(ht + 1) * P]
                                for (n0, nsz) in n_blocks:
                                    nc.tensor.matmul(
                                        ps[:, n0:n0 + nsz], lhsT,
                                        xn2Ts[gg][:, kt, n0:n0 + nsz],
                                        start=(kt == 0), stop=(kt == KD - 1),
                                    )
                            hidx = (2 * hbp + hb2) * KH + ht
                            nc.scalar.activation(
                                out=hT[:, hb2 * KH + ht, :], in_=ps[:, 0:T],
                                func=AF.Gelu, bias=cf1_sb[:, hidx:hidx + 1])
                    hTs.append(hT)
                for gg in range(GRP):
                    hT = hTs[gg]
                    for j, (t0, tsz) in enumerate(t_tiles):
                        ps = pmm.tile([P, D], F32, tag="mm")
                        for k8 in range(2 * KH):
                            lhsT = hT[:, k8, t0:t0 + tsz]
                            for o in range(2):
                                nc.tensor.matmul(
                                    ps[0:tsz, o * 512:(o + 1) * 512], lhsT,
                                    f2[:, k8 // KH, k8 % KH, o * 512:(o + 1) * 512],
                                    start=(k8 == 0), stop=(k8 == 2 * KH - 1),
                                )
                        x2s = x2s_t[gg][0:tsz, j, :]
                        nc.vector.tensor_add(out=x2s, in0=x2s, in1=ps[0:tsz, :])
            for gg in range(GRP):
                g = grp * GRP + gg
                for j, (t0, tsz) in enumerate(t_tiles):
                    nc.sync.dma_start(out=out_d[g * T + t0:g * T + t0 + tsz, :],
                                      in_=x2s_t[gg][0:tsz, j, :])

        # Interleaved emission: grp1's LN1 fills the PE gap at grp0's
        # C->D boundary (proj-evict -> stats -> norm chain on DVE).
        a0 = phase_a(0)
        aT0, wp0 = phase_b(0, a0)
        c0 = phase_c(0, aT0, wp0)
        a1 = phase_a(1)
        phase_d(0, *c0)
        aT1, wp1 = phase_b(1, a1)
        c1 = phase_c(1, aT1, wp1)
        phase_d(1, *c1)

    nc.compile()
    return nc


def prepare_inputs(inputs, B, T, D, H, HID, n_cores):
    """Host-side folding + weight permutation into DMA-friendly layouts."""
    f8 = np.float64
    bf16 = mybir.dt.np(BF)
    x = np.asarray(inputs["x"], np.float32)
    g1 = np.asarray(inputs["ln1_g"], f8)
    b1 = np.asarray(inputs["ln1_b"], f8)
    qkv_w = np.asarray(inputs["qkv_w"], f8)
    qkv_b = np.asarray(inputs["qkv_b"], f8)
    proj_w = np.asarray(inputs["proj_w"], np.float32)
    proj_b = np.asarray(inputs["proj_b"], np.float32)
    g2 = np.asarray(inputs["ln2_g"], f8)
    b2 = np.asarray(inputs["ln2_b"], f8)
    fc1_w = np.asarray(inputs["fc1_w"], f8)
    fc1_b = np.asarray(inputs["fc1_b"], f8)
    fc2_w = np.asarray(inputs["fc2_w"], np.float32)
    fc2_b = np.asarray(inputs["fc2_b"], np.float32)

    KD = D // P
    NPAIR = H // 2
    NHB = HID // 512
    KH = 512 // P
    NHT = HID // P

    wq = (g1[:, None] * qkv_w).astype(np.float32)
    cq = (b1 @ qkv_w + qkv_b).astype(np.float32)
    wq_, wk_, wv_ = wq[:, :D], wq[:, D:2 * D], wq[:, 2 * D:]
    Wq = np.stack([
        np.concatenate([wq_[:, p * P:(p + 1) * P], wk_[:, p * P:(p + 1) * P],
                        wv_[:, p * P:(p + 1) * P]], axis=1)
        for p in range(NPAIR)
    ], axis=0)                                            # [NPAIR, D, 384]
    wqkv_h = np.ascontiguousarray(
        Wq.reshape(NPAIR, KD, P, 3 * P).transpose(2, 0, 1, 3)).astype(bf16)
    cq_, ck_, cv_ = cq[:D], cq[D:2 * D], cq[2 * D:]
    cq_t = np.stack([cq_.reshape(NPAIR, P), ck_.reshape(NPAIR, P),
                     cv_.reshape(NPAIR, P)], axis=1)      # [NPAIR, 3, P]
    cq_t = np.ascontiguousarray(cq_t.transpose(2, 0, 1).reshape(P, NPAIR * 3),
                                dtype=np.float32)

    wproj_h = np.ascontiguousarray(
        proj_w.reshape(KD, P, D).transpose(1, 0, 2)).astype(bf16)
    wf1 = (g2[:, None] * fc1_w).astype(np.float32)
    cf1 = (b2 @ fc1_w + fc1_b).astype(np.float32)
    wf1_h = np.ascontiguousarray(
        wf1.reshape(KD, P, NHB, 512).transpose(1, 2, 0, 3)).astype(bf16)
    cf1_t = np.ascontiguousarray(cf1.reshape(NHT, P).T, dtype=np.float32)
    wf2_h = np.ascontiguousarray(
        fc2_w.reshape(NHB, KH, P, D).transpose(2, 0, 1, 3)).astype(bf16)
    cpb_h = np.ascontiguousarray(np.broadcast_to(proj_b, (P, D)), np.float32)
    cf2_h = np.ascontiguousarray(np.broadcast_to(fc2_b, (P, D)), np.float32)

    Bc = B // n_cores
    TOK = Bc * T
    shared = dict(wqkv=wqkv_h, cq=cq_t, wproj=wproj_h, cpb=cpb_h,
                  wfc1=wf1_h, cf1=cf1_t, wfc2=wf2_h, cf2=cf2_h)
    in_maps = []
    for c in range(n_cores):
        m = dict(shared)
        m["x"] = np.ascontiguousarray(x[c * Bc:(c + 1) * Bc].reshape(TOK, D))
        in_maps.append(m)
    return in_maps


_NC_CACHE = {}


def _get_nc(B, T, D, H, HD, HID):
    key = (B, T, D, H, HD, HID)
    if key not in _NC_CACHE:
        _NC_CACHE[key] = build_nc(B=B, T=T, D=D, H=H, HD=HD, HID=HID)
    return _NC_CACHE[key]


def _run(inputs, trace=False):
    from concourse.bass_utils import run_bass_kernel_spmd
    x = np.asarray(inputs["x"])
    B, T, D = x.shape
    H = 16
    HD = D // H
    HID = np.asarray(inputs["fc1_w"]).shape[1]
    n_cores = 8
    Bc = B // n_cores
    nc = _get_nc(Bc, T, D, H, HD, HID)
    in_maps = prepare_inputs(inputs, B, T, D, H, HID, n_cores)
    res = run_bass_kernel_spmd(nc, in_maps, list(range(n_cores)), trace=trace)
    out = np.concatenate(
        [res.results[c]["out"].reshape(Bc, T, D) for c in range(n_cores)], axis=0)
    return out, res


def kernel(**inputs) -> np.ndarray:
    out, _ = _run(inputs, trace=False)
    return out.astype(np.float32)
